# revision 13
# baseline (speedup 1.0000x reference)
"""DMPNN layer kernel for Trainium2 (8 NeuronCores, Bass/Tile).

Math (reference):
    direct   = cat([ef, nf[src]]) @ W + b          # [E, 64]
    backward = cat([ef, nf[dst]]) @ W + b          # [E, 64]
    full     = segment_sum(direct, dst, N)         # [N, 64]
    new_direct   = full[src] - backward
    new_backward = full[dst] - direct
    new_node     = relu(full)

Decomposition (W = [W_e; W_n], P = nf @ W_n + b, C = ef @ W_e):
    direct = C + P[src]; backward = C + P[dst]
    new_direct   = full[src] - C - P[dst]
    new_backward = full[dst] - C - P[src]

Sharding: edges split across 8 cores. Each core builds the node table
P into G_LO/G_HI (lo/hi halves; dma_gather indices are int16), segment-
sums its shard with dma_scatter_add into half-tables T_LO/T_HI using a
host-computed collision-free schedule (first occurrence of a dst within
each super keeps the row; extra occurrences go to dedicated aux rows and
are folded back by a few statically-sized spill rounds — dma_scatter_add
loses colliding updates within one call, so every call has unique rows),
AllReduces the halves, folds `full` into G, then a second edge pass
computes the outputs with dma_gather row gathers.
"""

import math
from dataclasses import dataclass

import numpy as np

import concourse.bass as bass
import concourse.bacc as bacc
import concourse.mybir as mybir
import concourse.tile as tile
from concourse.masks import make_identity

F32 = mybir.dt.float32
I16 = mybir.dt.int16


@dataclass(frozen=True)
class Cfg:
    n_nodes: int = 50000
    n_edges: int = 800000
    n_cores: int = 8
    supb: int = 4096                    # edges per phase-B super
    supc: int = 2048                    # edges per phase-C super
    spill_caps: tuple = (1024, 512, 256, 128, 128)
    collective: bool = True

    @property
    def ec(self):
        assert self.n_edges % self.n_cores == 0
        return self.n_edges // self.n_cores

    @property
    def nsupb(self):
        return math.ceil(self.ec / self.supb)

    @property
    def ecp(self):
        return self.nsupb * self.supb

    @property
    def nsupc(self):
        assert self.ecp % self.supc == 0
        return self.ecp // self.supc

    @property
    def nt(self):                       # node rows incl pad row n_nodes
        return (math.ceil((self.n_nodes + 1) / 128)) * 128

    @property
    def hsplit(self):
        return math.ceil(self.n_nodes / 2 / 512) * 512

    @property
    def aux(self):
        return sum(self.spill_caps)

    # scatter tables: [0,half) nodes | 128 trash | aux spill rows
    @property
    def tlo_rows(self):
        return self.hsplit + 128 + self.aux

    @property
    def thi_rows(self):
        return (self.nt - self.hsplit) + 128 + self.aux

    # gather tables: row 0 = zeros, rows 1.. = nodes
    @property
    def glo_rows(self):
        return self.hsplit + 128

    @property
    def ghi_rows(self):
        return (self.nt - self.hsplit) + 128


def build_program(cfg: Cfg) -> bass.Bass:
    N, NT, HS = cfg.n_nodes, cfg.nt, cfg.hsplit
    SUPB, KJB, NSUPB = cfg.supb, cfg.supb // 128, cfg.nsupb
    SUPC, KJC, NSUPC = cfg.supc, cfg.supc // 128, cfg.nsupc
    ECP = cfg.ecp
    OF, IF, EFD = 64, 128, 64
    assert HS % 512 == 0
    for t in (cfg.tlo_rows, cfg.thi_rows, cfg.glo_rows, cfg.ghi_rows):
        assert t <= 32768, t

    nc = bacc.Bacc(None, target_bir_lowering=False)

    # ---- I/O -----------------------------------------------------------
    nf = nc.dram_tensor("node_feats", [N, IF], F32, kind="ExternalInput")
    W = nc.dram_tensor("W", [IF + EFD, OF], F32, kind="ExternalInput")
    b4 = nc.dram_tensor("b4", [128, 4 * OF], F32, kind="ExternalInput")
    ef = nc.dram_tensor("ef", [ECP, EFD], F32, kind="ExternalInput")
    sBlo = nc.dram_tensor("sBlo", [128, ECP // 16], I16, kind="ExternalInput")
    sBhi = nc.dram_tensor("sBhi", [128, ECP // 16], I16, kind="ExternalInput")
    sClo = nc.dram_tensor("sClo", [128, ECP // 16], I16, kind="ExternalInput")
    sChi = nc.dram_tensor("sChi", [128, ECP // 16], I16, kind="ExternalInput")
    dClo = nc.dram_tensor("dClo", [128, ECP // 16], I16, kind="ExternalInput")
    dChi = nc.dram_tensor("dChi", [128, ECP // 16], I16, kind="ExternalInput")
    mMlo = nc.dram_tensor("mMlo", [128, ECP // 16], I16, kind="ExternalInput")
    mMhi = nc.dram_tensor("mMhi", [128, ECP // 16], I16, kind="ExternalInput")
    spills = {}
    for h in ("lo", "hi"):
        for r, cap in enumerate(cfg.spill_caps):
            spills[(h, r)] = nc.dram_tensor(
                f"sp_{h}_{r}", [128, cap // 16], I16, kind="ExternalInput")

    new_direct = nc.dram_tensor("new_direct", [ECP, OF], F32, kind="ExternalOutput")
    new_backward = nc.dram_tensor("new_backward", [ECP, OF], F32, kind="ExternalOutput")
    new_node = nc.dram_tensor("new_node", [N, OF], F32, kind="ExternalOutput")

    # ---- internal DRAM -------------------------------------------------
    GLO = nc.dram_tensor("GLO", [cfg.glo_rows, 2 * OF], F32)
    GHI = nc.dram_tensor("GHI", [cfg.ghi_rows, 2 * OF], F32)
    TLO = nc.dram_tensor("TLO", [cfg.tlo_rows, OF], F32)
    THI = nc.dram_tensor("THI", [cfg.thi_rows, OF], F32)
    if cfg.collective:
        aspace = "Shared" if cfg.n_cores > 4 else "Local"
        SLO = nc.dram_tensor("SLO", [HS, OF], F32, addr_space=aspace)
        SHI = nc.dram_tensor("SHI", [NT - HS, OF], F32, addr_space=aspace)
    else:
        SLO = nc.dram_tensor("SLO", [HS, OF], F32)
        SHI = nc.dram_tensor("SHI", [NT - HS, OF], F32)

    rgroups = [list(range(cfg.n_cores))]

    def zero_chunks(view_fn, rows, width_elems, zeros):
        r = 0
        while r < rows:
            step = min(2048, rows - r)
            p = step // 16 if step >= 16 else step
            while step % p:
                p -= 1
            k = step // p
            nc.sync.dma_start(
                out=view_fn(r, step, p),
                in_=zeros[:p, :k * width_elems],
            )
            r += step

    with tile.TileContext(nc) as tc:
        with tc.tile_pool(name="const", bufs=1) as cpool:
            ident = cpool.tile([128, 128], F32, tag="ident")
            make_identity(nc, ident[:])
            wbd = cpool.tile([128, 128], F32, tag="wbd")
            nc.gpsimd.memset(wbd[:], 0.0)
            nc.sync.dma_start(out=wbd[0:64, 0:64], in_=W[0:EFD, :])
            nc.sync.dma_start(out=wbd[64:128, 64:128], in_=W[0:EFD, :])
            wn = cpool.tile([128, OF], F32, tag="wn")
            nc.sync.dma_start(out=wn[:], in_=W[EFD:, :])
            bb = cpool.tile([128, 4 * OF], F32, tag="bb")
            nc.sync.dma_start(out=bb[:], in_=b4[:])
            zeros = cpool.tile([128, 2048], F32, tag="zeros")
            nc.gpsimd.memset(zeros[:], 0.0)

            # zero scatter tables, G `full` stripes, G zero rows
            for tab in (TLO, THI):
                zero_chunks(
                    lambda r, s, p, tab=tab: tab[r:r + s, :].rearrange(
                        "(p k) f -> p k f", p=p),
                    tab.shape[0], OF, zeros)
            for g in (GLO, GHI):
                zero_chunks(
                    lambda r, s, p, g=g: g[r:r + s, OF:].rearrange(
                        "(p k) f -> p k f", p=p),
                    g.shape[0], OF, zeros)
                zero_chunks(
                    lambda r, s, p, g=g: g[r:r + s, 0:OF].rearrange(
                        "(p k) f -> p k f", p=p),
                    g.shape[0], OF, zeros)

            # ---- phase P: G*[1+n, 0:64] = nf @ W_n + b -----------------
            with (
                tc.tile_pool(name="ppool", bufs=3) as pp,
                tc.tile_pool(name="ppsum", bufs=3, space="PSUM") as pps,
            ):
                ngroups = math.ceil(N / 512)
                for g in range(ngroups):
                    r0 = g * 512
                    rows = min(512, N - r0)
                    nt_in = pp.tile([128, 4 * IF], F32, tag="nfin")
                    if rows < 512:
                        nc.gpsimd.memset(nt_in[:], 0.0)
                    pcnt = rows // 4
                    nc.sync.dma_start(
                        out=nt_in[:pcnt, :],
                        in_=nf[r0:r0 + rows, :].rearrange("(p k) f -> p k f", p=pcnt),
                    )
                    pch = pps.tile([128, 4 * OF], F32, tag="pchunk", space="PSUM")
                    for k in range(4):
                        tp = pps.tile([128, 128], F32, tag="ptp", space="PSUM")
                        nc.tensor.transpose(
                            out=tp[:], in_=nt_in[:, k * IF:(k + 1) * IF],
                            identity=ident[:])
                        tps = pp.tile([128, 128], F32, tag="ptps")
                        nc.vector.tensor_copy(out=tps[:], in_=tp[:])
                        nc.tensor.matmul(
                            out=pch[:, k * OF:(k + 1) * OF],
                            lhsT=tps[:], rhs=wn[:], start=True, stop=True)
                    pout = pp.tile([128, 4 * OF], F32, tag="pout")
                    nc.vector.tensor_tensor(
                        out=pout[:], in0=pch[:], in1=bb[:], op=mybir.AluOpType.add)
                    if r0 < HS:
                        gt, gr = GLO, 1 + r0
                    else:
                        gt, gr = GHI, 1 + (r0 - HS)
                    wrows = min(512, gt.shape[0] - gr)
                    wrows -= wrows % 4
                    wp = wrows // 4
                    nc.sync.dma_start(
                        out=gt[gr:gr + wrows, 0:OF].rearrange(
                            "(p k) f -> p k f", p=wp),
                        in_=pout[:wp, :])

            # ---- phase B: direct = C + P[src]; scatter-add -------------
            with (
                tc.tile_pool(name="bpool", bufs=2) as bp,
                tc.tile_pool(name="bpsum", bufs=2, space="PSUM") as bps,
                tc.tile_pool(name="bqpsum", bufs=4, space="PSUM") as bqs,
            ):
                nq = (KJB * OF) // 512
                ncol = SUPB // 16
                for s in range(NSUPB):
                    e0 = s * SUPB
                    eft = bp.tile([128, KJB * EFD], F32, tag="eft")
                    nc.sync.dma_start(
                        out=eft[:],
                        in_=ef[e0:e0 + SUPB, :].rearrange("(p j) f -> p j f", p=128))
                    efT = bp.tile([128, KJB * EFD], F32, tag="efT")
                    for m in range(KJB // 2):
                        tp = bps.tile([128, 128], F32, tag="btp", space="PSUM")
                        nc.tensor.transpose(
                            out=tp[:], in_=eft[:, m * 128:(m + 1) * 128],
                            identity=ident[:])
                        nc.vector.tensor_copy(
                            out=efT[:, m * 128:(m + 1) * 128], in_=tp[:])
                    pslo = bp.tile([128, KJB * OF], F32, tag="pslo")
                    pshi = bp.tile([128, KJB * OF], F32, tag="pshi")
                    for t, src_idx, gt, tg in (
                        (pslo, sBlo, GLO, "iblo"), (pshi, sBhi, GHI, "ibhi")):
                        it = bp.tile([128, ncol], I16, tag=tg)
                        nc.sync.dma_start(
                            out=it[:], in_=src_idx[:, s * ncol:(s + 1) * ncol])
                        t3 = t[:].rearrange("p (t f) -> p t f", f=OF)
                        for c in range(SUPB // 1024):
                            nc.gpsimd.dma_gather(
                                t3[:, c * 8:(c + 1) * 8, :],
                                gt[:, 0:OF], it[:, c * 64:(c + 1) * 64],
                                1024, 1024, OF, elem_step=2 * OF)
                    ps = pslo
                    nc.vector.tensor_tensor(
                        out=ps[:], in0=pslo[:], in1=pshi[:], op=mybir.AluOpType.add)
                    direct = bp.tile([128, KJB * OF], F32, tag="direct")
                    d3 = direct[:].rearrange("p (t f) -> p t f", f=OF)
                    ps3 = ps[:].rearrange("p (t f) -> p t f", f=OF)
                    for q in range(nq):
                        cq = bqs.tile([128, 512], F32, tag="cq", space="PSUM")
                        for m in range(4):
                            jp = q * 4 + m
                            nc.tensor.matmul(
                                out=cq[:, m * 128:(m + 1) * 128],
                                lhsT=efT[:, jp * 128:(jp + 1) * 128],
                                rhs=wbd[:], start=True, stop=True)
                        cq3 = cq[:].rearrange("p (t f) -> p t f", f=OF)
                        nc.vector.tensor_tensor(
                            out=d3[:, q * 8:(q + 1) * 8, :], in0=cq3[:],
                            in1=ps3[:, q * 8:(q + 1) * 8, :], op=mybir.AluOpType.add)
                    for tab, idx_src, tag in ((TLO, mMlo, "imlo"), (THI, mMhi, "imhi")):
                        it = bp.tile([128, ncol], I16, tag=tag)
                        nc.sync.dma_start(
                            out=it[:], in_=idx_src[:, s * ncol:(s + 1) * ncol])
                        for c in range(SUPB // 1024):
                            nc.gpsimd.dma_scatter_add(
                                tab[:], d3[:, c * 8:(c + 1) * 8, :],
                                it[:, c * 64:(c + 1) * 64], 1024, 1024, OF)

                # spill rounds: fold aux rows back into their true rows
                AUX = cfg.aux
                for tab, h in ((TLO, "lo"), (THI, "hi")):
                    aux_base = tab.shape[0] - AUX
                    at = bp.tile([128, (AUX // 128) * OF], F32, tag="aux" + h)
                    nc.sync.dma_start(
                        out=at[:].rearrange("p (k f) -> p k f", f=OF),
                        in_=tab[aux_base:, :].rearrange("(k p) f -> p k f", p=128))
                    a3 = at[:].rearrange("p (t f) -> p t f", f=OF)
                    off = 0
                    for r, cap in enumerate(cfg.spill_caps):
                        it = bp.tile([128, cap // 16], I16, tag=f"isp{h}{r}")
                        nc.sync.dma_start(out=it[:], in_=spills[(h, r)][:])
                        co = 0
                        while co < cap:
                            step = min(1024, cap - co)
                            nc.gpsimd.dma_scatter_add(
                                tab[:],
                                a3[:, (off + co) // 128:(off + co + step) // 128, :],
                                it[:, co // 16:(co + step) // 16],
                                step, step, OF)
                            co += step
                        off += cap

            # ---- AllReduce the two halves (real node rows only) --------
            if cfg.collective:
                nc.gpsimd.collective_compute(
                    "AllReduce", mybir.AluOpType.add,
                    ins=[TLO[0:HS, :]], outs=[SLO[:]], replica_groups=rgroups)
                nc.gpsimd.collective_compute(
                    "AllReduce", mybir.AluOpType.add,
                    ins=[THI[0:NT - HS, :]], outs=[SHI[:]], replica_groups=rgroups)
            else:
                for stab, tab, rows in ((SLO, TLO, HS), (SHI, THI, NT - HS)):
                    zc = 0
                    while zc < rows:
                        st = min(2048, rows - zc)
                        p = st // 16
                        nc.sync.dma_start(
                            out=stab[zc:zc + st, :].rearrange(
                                "(p k) f -> p k f", p=p),
                            in_=tab[zc:zc + st, :].rearrange(
                                "(p k) f -> p k f", p=p))
                        zc += st

            # ---- fold full into G + new_node = relu(full) --------------
            with tc.tile_pool(name="rpool", bufs=3) as rp:
                for half, (stab, gtab, base) in enumerate(
                        ((SLO, GLO, 0), (SHI, GHI, HS))):
                    real = HS if half == 0 else (N - HS)
                    r = 0
                    while r < real:
                        left = real - r
                        if left >= 16:
                            step = min(2048, left - left % 16)
                            p = step // 16
                        else:
                            step, p = left, left
                        w = (step // p) * OF
                        ft = rp.tile([128, 1024], F32, tag="fchunk")
                        nc.sync.dma_start(
                            out=ft[:p, :w],
                            in_=stab[r:r + step, :].rearrange(
                                "(p k) f -> p k f", p=p))
                        nc.sync.dma_start(
                            out=gtab[1 + r:1 + r + step, OF:].rearrange(
                                "(p k) f -> p k f", p=p),
                            in_=ft[:p, :w])
                        rl = rp.tile([128, 1024], F32, tag="rchunk")
                        nc.scalar.activation(
                            out=rl[:p, :w], in_=ft[:p, :w],
                            func=mybir.ActivationFunctionType.Relu)
                        nc.sync.dma_start(
                            out=new_node[base + r:base + r + step, :].rearrange(
                                "(p k) f -> p k f", p=p),
                            in_=rl[:p, :w])
                        r += step

            # ---- phase C: outputs --------------------------------------
            with (
                tc.tile_pool(name="cpool", bufs=2) as cp,
                tc.tile_pool(name="cpsum", bufs=2, space="PSUM") as cps,
                tc.tile_pool(name="cqpsum", bufs=4, space="PSUM") as cqs,
            ):
                nq = (KJC * OF) // 512
                ncol = SUPC // 16
                for s in range(NSUPC):
                    e0 = s * SUPC
                    eft = cp.tile([128, KJC * EFD], F32, tag="ceft")
                    nc.sync.dma_start(
                        out=eft[:],
                        in_=ef[e0:e0 + SUPC, :].rearrange("(p j) f -> p j f", p=128))
                    efT = cp.tile([128, KJC * EFD], F32, tag="cefT")
                    for m in range(KJC // 2):
                        tp = cps.tile([128, 128], F32, tag="ctp", space="PSUM")
                        nc.tensor.transpose(
                            out=tp[:], in_=eft[:, m * 128:(m + 1) * 128],
                            identity=ident[:])
                        nc.vector.tensor_copy(
                            out=efT[:, m * 128:(m + 1) * 128], in_=tp[:])
                    gath = {}
                    for nm, idx_src, gt in (
                        ("slo", sClo, GLO), ("shi", sChi, GHI),
                        ("dlo", dClo, GLO), ("dhi", dChi, GHI),
                    ):
                        t = cp.tile([128, KJC * 2 * OF], F32, tag="g" + nm)
                        it = cp.tile([128, ncol], I16, tag="i" + nm)
                        nc.sync.dma_start(
                            out=it[:], in_=idx_src[:, s * ncol:(s + 1) * ncol])
                        t3 = t[:].rearrange("p (t f) -> p t f", f=2 * OF)
                        for c in range(SUPC // 1024):
                            nc.gpsimd.dma_gather(
                                t3[:, c * 8:(c + 1) * 8, :],
                                gt[:], it[:, c * 64:(c + 1) * 64],
                                1024, 1024, 2 * OF)
                        gath[nm] = t
                    gs, gd = gath["slo"], gath["dlo"]
                    nc.vector.tensor_tensor(
                        out=gs[:], in0=gath["slo"][:], in1=gath["shi"][:],
                        op=mybir.AluOpType.add)
                    nc.vector.tensor_tensor(
                        out=gd[:], in0=gath["dlo"][:], in1=gath["dhi"][:],
                        op=mybir.AluOpType.add)
                    gs3 = gs[:].rearrange("p (t f) -> p t f", f=2 * OF)
                    gd3 = gd[:].rearrange("p (t f) -> p t f", f=2 * OF)
                    nd = cp.tile([128, KJC * OF], F32, tag="nd")
                    nb = cp.tile([128, KJC * OF], F32, tag="nb")
                    nd3 = nd[:].rearrange("p (t f) -> p t f", f=OF)
                    nb3 = nb[:].rearrange("p (t f) -> p t f", f=OF)
                    for q in range(nq):
                        cq = cqs.tile([128, 512], F32, tag="ccq", space="PSUM")
                        for m in range(4):
                            jp = q * 4 + m
                            nc.tensor.matmul(
                                out=cq[:, m * 128:(m + 1) * 128],
                                lhsT=efT[:, jp * 128:(jp + 1) * 128],
                                rhs=wbd[:], start=True, stop=True)
                        cq3 = cq[:].rearrange("p (t f) -> p t f", f=OF)
                        sl = slice(q * 8, (q + 1) * 8)
                        nc.vector.tensor_tensor(
                            out=nd3[:, sl, :], in0=gs3[:, sl, OF:],
                            in1=cq3[:], op=mybir.AluOpType.subtract)
                        nc.vector.tensor_tensor(
                            out=nd3[:, sl, :], in0=nd3[:, sl, :],
                            in1=gd3[:, sl, 0:OF], op=mybir.AluOpType.subtract)
                        nc.vector.tensor_tensor(
                            out=nb3[:, sl, :], in0=gd3[:, sl, OF:],
                            in1=cq3[:], op=mybir.AluOpType.subtract)
                        nc.vector.tensor_tensor(
                            out=nb3[:, sl, :], in0=nb3[:, sl, :],
                            in1=gs3[:, sl, 0:OF], op=mybir.AluOpType.subtract)
                    nc.sync.dma_start(
                        out=new_direct[e0:e0 + SUPC, :].rearrange(
                            "(p j) f -> p j f", p=128),
                        in_=nd[:])
                    nc.sync.dma_start(
                        out=new_backward[e0:e0 + SUPC, :].rearrange(
                            "(p j) f -> p j f", p=128),
                        in_=nb[:])

    nc.compile()
    return nc


# ======================= host-side preparation =========================

CALL = 1024                 # tokens per dma_gather/dma_scatter_add call


def _tok_order(S):
    """token q -> edge offset within a super of size S, 1024-token calls.

    Call chunk c covers j-blocks [8c, 8c+8); within a chunk token
    u = jj*128 + p maps to edge p*(S//128) + 8c + jj."""
    q = np.arange(S)
    c, r = q // CALL, q % CALL
    jj, p = r // 128, r % 128
    return p * (S // 128) + c * 8 + jj


def _wrap16(a):
    """[NSUP, S] token-ordered values -> [128, total//16] int16 wrapped."""
    ns, S = a.shape
    w = a.reshape(ns, S // 16, 16).swapaxes(1, 2).reshape(ns, 16, S // 16)
    w = np.concatenate(list(w), axis=1)
    return np.tile(w.astype(np.int16), (8, 1)).copy()


def prep_core_inputs(cfg: Cfg, node_feats, W, b, ef_shard, src_shard, dst_shard):
    N, HS, NT = cfg.n_nodes, cfg.hsplit, cfg.nt
    SUPB, SUPC, ECP, EC = cfg.supb, cfg.supc, cfg.ecp, len(src_shard)

    efp = np.zeros((ECP, 64), dtype=np.float32)
    efp[:EC] = ef_shard
    srcp = np.full((ECP,), N, dtype=np.int64)
    srcp[:EC] = src_shard
    dstp = np.full((ECP,), N, dtype=np.int64)
    dstp[:EC] = dst_shard

    def gather_idx(vals, S):
        tok = vals.reshape(-1, S)[:, _tok_order(S)]
        lo = np.where(tok < HS, tok + 1, 0).reshape(-1, CALL)
        hi = np.where(tok >= HS, tok - HS + 1, 0).reshape(-1, CALL)
        return _wrap16(lo), _wrap16(hi)

    sBlo, sBhi = gather_idx(srcp, SUPB)
    sClo, sChi = gather_idx(srcp, SUPC)
    dClo, dChi = gather_idx(dstp, SUPC)

    # ---- collision-free scatter schedule (phase-B token order) ---------
    NSUPB = cfg.nsupb
    dst_tok = dstp.reshape(NSUPB, SUPB)[:, _tok_order(SUPB)]
    flat = dst_tok.ravel()
    valid = flat < N                      # pad tokens go straight to trash
    n = flat.size
    sup_id = np.repeat(np.arange(n // CALL), CALL)   # per-call dedup
    key_d = np.where(valid, flat, -1)
    order = np.lexsort((np.arange(n), key_d, sup_id))
    sd, ss = key_d[order], sup_id[order]
    newg = np.r_[True, (ss[1:] != ss[:-1]) | (sd[1:] != sd[:-1])]
    pos = np.arange(n)
    rank_sorted = pos - np.maximum.accumulate(np.where(newg, pos, 0))
    rank = np.empty(n, np.int64)
    rank[order] = rank_sorted
    spill_mask = valid & (rank > 0)

    # global spill round per dst
    sp_pos = np.nonzero(spill_mask)[0]
    sp_dst = flat[sp_pos]
    o2 = np.lexsort((sp_pos, sp_dst))
    ng2 = np.r_[True, sp_dst[o2][1:] != sp_dst[o2][:-1]]
    p2 = np.arange(sp_pos.size)
    r2 = p2 - np.maximum.accumulate(np.where(ng2, p2, 0))
    round_of = np.empty(sp_pos.size, np.int64)
    round_of[o2] = r2

    caps = cfg.spill_caps
    if sp_pos.size and round_of.max() >= len(caps):
        raise RuntimeError(
            f"spill rounds overflow: need {round_of.max() + 1} > {len(caps)}")

    thi_node_rows = NT - HS
    trash_lo = HS + (np.arange(n) % 128)
    trash_hi = thi_node_rows + (np.arange(n) % 128)
    mlo = np.where(valid & (flat < HS) & (rank == 0), flat, trash_lo)
    mhi = np.where(valid & (flat >= HS) & (rank == 0), flat - HS, trash_hi)
    aux_lo_base = HS + 128
    aux_hi_base = thi_node_rows + 128
    cap_off = np.concatenate([[0], np.cumsum(caps)])
    sp_half_hi = sp_dst >= HS
    sp_arrays = {}
    for h, hi_sel, aux_base, node_base in (
        ("lo", False, aux_lo_base, 0), ("hi", True, aux_hi_base, HS)):
        for r, cap in enumerate(caps):
            sel = (sp_half_hi == hi_sel) & (round_of == r)
            cnt = int(sel.sum())
            if cnt > cap:
                raise RuntimeError(
                    f"spill capacity overflow: half {h} round {r}: {cnt} > {cap}")
            slots = np.nonzero(sel)[0]
            aux_rel = cap_off[r] + np.arange(cnt)
            if h == "lo":
                mlo[sp_pos[slots]] = aux_base + aux_rel
            else:
                mhi[sp_pos[slots]] = aux_base + aux_rel
            arr = (aux_base + cap_off[r] + np.arange(cap)).astype(np.int64)
            arr[:cnt] = sp_dst[slots] - node_base
            sp_arrays[(h, r)] = _wrap16(arr.reshape(1, cap))

    assert mlo.max() < cfg.tlo_rows and mhi.max() < cfg.thi_rows

    out = {
        "node_feats": np.ascontiguousarray(node_feats, dtype=np.float32),
        "W": np.ascontiguousarray(W, dtype=np.float32),
        "b4": np.tile(b.astype(np.float32), (128, 4)).copy(),
        "ef": efp,
        "sBlo": sBlo, "sBhi": sBhi,
        "sClo": sClo, "sChi": sChi, "dClo": dClo, "dChi": dChi,
        "mMlo": _wrap16(mlo.reshape(-1, CALL)),
        "mMhi": _wrap16(mhi.reshape(-1, CALL)),
    }
    for (h, r), arr in sp_arrays.items():
        out[f"sp_{h}_{r}"] = arr
    return out


_PROG_CACHE: dict = {}


def _get_program(cfg: Cfg) -> bass.Bass:
    if cfg not in _PROG_CACHE:
        _PROG_CACHE[cfg] = build_program(cfg)
    return _PROG_CACHE[cfg]


def kernel(node_feats, edge_feats, W, b, src, dst, _trace=False):
    from concourse.bass_utils import run_bass_kernel_spmd

    node_feats = np.asarray(node_feats)
    edge_feats = np.asarray(edge_feats)
    W = np.asarray(W)
    b = np.asarray(b)
    src = np.asarray(src)
    dst = np.asarray(dst)

    cfg = Cfg(n_nodes=node_feats.shape[0], n_edges=edge_feats.shape[0])
    nc = _get_program(cfg)
    ec = cfg.ec
    in_maps = [
        prep_core_inputs(
            cfg, node_feats, W, b,
            edge_feats[c * ec:(c + 1) * ec],
            src[c * ec:(c + 1) * ec],
            dst[c * ec:(c + 1) * ec],
        )
        for c in range(cfg.n_cores)
    ]
    res = run_bass_kernel_spmd(
        nc, in_maps, core_ids=list(range(cfg.n_cores)), trace=_trace)
    outs = res.results
    new_direct = np.concatenate(
        [outs[c]["new_direct"][:ec] for c in range(cfg.n_cores)])
    new_backward = np.concatenate(
        [outs[c]["new_backward"][:ec] for c in range(cfg.n_cores)])
    new_node = outs[0]["new_node"]
    kernel.last_results = res
    return (new_node, new_direct, new_backward)


# revision 14
# speedup vs baseline: 1.3242x; 1.3242x over previous
"""DMPNN layer kernel for Trainium2 (8 NeuronCores, Bass/Tile).

Math (reference):
    direct   = cat([ef, nf[src]]) @ W + b          # [E, 64]
    backward = cat([ef, nf[dst]]) @ W + b          # [E, 64]
    full     = segment_sum(direct, dst, N)         # [N, 64]
    new_direct   = full[src] - backward
    new_backward = full[dst] - direct
    new_node     = relu(full)

Decomposition (W = [W_e; W_n], P = nf @ W_n + b, C = ef @ W_e):
    direct = C + P[src]; backward = C + P[dst]
    new_direct   = full[src] - C - P[dst]
    new_backward = full[dst] - C - P[src]

Sharding: edges split across 8 cores. Each core builds the node table
P into G_LO/G_HI (lo/hi halves; dma_gather indices are int16), segment-
sums its shard with dma_scatter_add into half-tables T_LO/T_HI using a
host-computed collision-free schedule (first occurrence of a dst within
each super keeps the row; extra occurrences go to dedicated aux rows and
are folded back by a few statically-sized spill rounds — dma_scatter_add
loses colliding updates within one call, so every call has unique rows),
AllReduces the halves, folds `full` into G, then a second edge pass
computes the outputs with dma_gather row gathers.
"""

import math
from dataclasses import dataclass

import numpy as np

import concourse.bass as bass
import concourse.bacc as bacc
import concourse.mybir as mybir
import concourse.tile as tile
from concourse.masks import make_identity

F32 = mybir.dt.float32
I16 = mybir.dt.int16


@dataclass(frozen=True)
class Cfg:
    n_nodes: int = 50000
    n_edges: int = 800000
    n_cores: int = 8
    supb: int = 4096                    # edges per phase-B super
    supc: int = 2048                    # edges per phase-C super
    spill_caps: tuple = (1024, 512, 256, 128, 128)
    collective: bool = True

    @property
    def ec(self):
        assert self.n_edges % self.n_cores == 0
        return self.n_edges // self.n_cores

    @property
    def nsupb(self):
        return math.ceil(self.ec / self.supb)

    @property
    def ecp(self):
        return self.nsupb * self.supb

    @property
    def nsupc(self):
        assert self.ecp % self.supc == 0
        return self.ecp // self.supc

    @property
    def nt(self):                       # node rows incl pad row n_nodes
        return (math.ceil((self.n_nodes + 1) / 128)) * 128

    @property
    def hsplit(self):
        return math.ceil(self.n_nodes / 2 / 512) * 512

    @property
    def aux(self):
        return sum(self.spill_caps)

    # scatter tables: [0,half) nodes | 128 trash | aux spill rows
    @property
    def tlo_rows(self):
        return self.hsplit + 128 + self.aux

    @property
    def thi_rows(self):
        return (self.nt - self.hsplit) + 128 + self.aux

    # gather tables: row 0 = zeros, rows 1.. = nodes
    @property
    def glo_rows(self):
        return self.hsplit + 128

    @property
    def ghi_rows(self):
        return (self.nt - self.hsplit) + 128


def build_program(cfg: Cfg) -> bass.Bass:
    N, NT, HS = cfg.n_nodes, cfg.nt, cfg.hsplit
    SUPB, KJB, NSUPB = cfg.supb, cfg.supb // 128, cfg.nsupb
    SUPC, KJC, NSUPC = cfg.supc, cfg.supc // 128, cfg.nsupc
    ECP = cfg.ecp
    OF, IF, EFD = 64, 128, 64
    assert HS % 512 == 0
    for t in (cfg.tlo_rows, cfg.thi_rows, cfg.glo_rows, cfg.ghi_rows):
        assert t <= 32768, t

    nc = bacc.Bacc(None, target_bir_lowering=False)

    # ---- I/O -----------------------------------------------------------
    nf = nc.dram_tensor("node_feats", [N, IF], F32, kind="ExternalInput")
    W = nc.dram_tensor("W", [IF + EFD, OF], F32, kind="ExternalInput")
    b4 = nc.dram_tensor("b4", [128, 4 * OF], F32, kind="ExternalInput")
    ef = nc.dram_tensor("ef", [ECP, EFD], F32, kind="ExternalInput")
    sBlo = nc.dram_tensor("sBlo", [128, ECP // 16], I16, kind="ExternalInput")
    sBhi = nc.dram_tensor("sBhi", [128, ECP // 16], I16, kind="ExternalInput")
    sClo = nc.dram_tensor("sClo", [128, ECP // 16], I16, kind="ExternalInput")
    sChi = nc.dram_tensor("sChi", [128, ECP // 16], I16, kind="ExternalInput")
    dClo = nc.dram_tensor("dClo", [128, ECP // 16], I16, kind="ExternalInput")
    dChi = nc.dram_tensor("dChi", [128, ECP // 16], I16, kind="ExternalInput")
    mMlo = nc.dram_tensor("mMlo", [128, ECP // 16], I16, kind="ExternalInput")
    mMhi = nc.dram_tensor("mMhi", [128, ECP // 16], I16, kind="ExternalInput")
    spills = {}
    for h in ("lo", "hi"):
        for r, cap in enumerate(cfg.spill_caps):
            spills[(h, r)] = nc.dram_tensor(
                f"sp_{h}_{r}", [128, cap // 16], I16, kind="ExternalInput")

    new_direct = nc.dram_tensor("new_direct", [ECP, OF], F32, kind="ExternalOutput")
    new_backward = nc.dram_tensor("new_backward", [ECP, OF], F32, kind="ExternalOutput")
    new_node = nc.dram_tensor("new_node", [N, OF], F32, kind="ExternalOutput")

    # ---- internal DRAM -------------------------------------------------
    GLO = nc.dram_tensor("GLO", [cfg.glo_rows, 2 * OF], F32)
    GHI = nc.dram_tensor("GHI", [cfg.ghi_rows, 2 * OF], F32)
    TLO = nc.dram_tensor("TLO", [cfg.tlo_rows, OF], F32)
    THI = nc.dram_tensor("THI", [cfg.thi_rows, OF], F32)
    if cfg.collective:
        aspace = "Shared" if cfg.n_cores > 4 else "Local"
        SLO = nc.dram_tensor("SLO", [HS, OF], F32, addr_space=aspace)
        SHI = nc.dram_tensor("SHI", [NT - HS, OF], F32, addr_space=aspace)
    else:
        SLO = nc.dram_tensor("SLO", [HS, OF], F32)
        SHI = nc.dram_tensor("SHI", [NT - HS, OF], F32)

    rgroups = [list(range(cfg.n_cores))]

    def zero_chunks(view_fn, rows, width_elems, zeros):
        r = 0
        while r < rows:
            step = min(2048, rows - r)
            p = step // 16 if step >= 16 else step
            while step % p:
                p -= 1
            k = step // p
            nc.sync.dma_start(
                out=view_fn(r, step, p),
                in_=zeros[:p, :k * width_elems],
            )
            r += step

    with tile.TileContext(nc) as tc:
        with tc.tile_pool(name="const", bufs=1) as cpool:
            ident = cpool.tile([128, 128], F32, tag="ident")
            make_identity(nc, ident[:])
            wbd = cpool.tile([128, 128], F32, tag="wbd")
            nc.gpsimd.memset(wbd[:], 0.0)
            nc.sync.dma_start(out=wbd[0:64, 0:64], in_=W[0:EFD, :])
            nc.sync.dma_start(out=wbd[64:128, 64:128], in_=W[0:EFD, :])
            wn = cpool.tile([128, OF], F32, tag="wn")
            nc.sync.dma_start(out=wn[:], in_=W[EFD:, :])
            bb = cpool.tile([128, 4 * OF], F32, tag="bb")
            nc.sync.dma_start(out=bb[:], in_=b4[:])
            zeros = cpool.tile([128, 2048], F32, tag="zeros")
            nc.gpsimd.memset(zeros[:], 0.0)

            # zero scatter tables, G `full` stripes, G zero rows
            for tab in (TLO, THI):
                zero_chunks(
                    lambda r, s, p, tab=tab: tab[r:r + s, :].rearrange(
                        "(p k) f -> p k f", p=p),
                    tab.shape[0], OF, zeros)
            for g in (GLO, GHI):
                zero_chunks(
                    lambda r, s, p, g=g: g[r:r + s, OF:].rearrange(
                        "(p k) f -> p k f", p=p),
                    g.shape[0], OF, zeros)
                zero_chunks(
                    lambda r, s, p, g=g: g[r:r + s, 0:OF].rearrange(
                        "(p k) f -> p k f", p=p),
                    g.shape[0], OF, zeros)

            # ---- phase P: G*[1+n, 0:64] = nf @ W_n + b -----------------
            with (
                tc.tile_pool(name="ppool", bufs=3) as pp,
                tc.tile_pool(name="ppsum", bufs=3, space="PSUM") as pps,
            ):
                ngroups = math.ceil(N / 512)
                for g in range(ngroups):
                    r0 = g * 512
                    rows = min(512, N - r0)
                    nt_in = pp.tile([128, 4 * IF], F32, tag="nfin")
                    if rows < 512:
                        nc.gpsimd.memset(nt_in[:], 0.0)
                    pcnt = rows // 4
                    nc.sync.dma_start(
                        out=nt_in[:pcnt, :],
                        in_=nf[r0:r0 + rows, :].rearrange("(p k) f -> p k f", p=pcnt),
                    )
                    pch = pps.tile([128, 4 * OF], F32, tag="pchunk", space="PSUM")
                    for k in range(4):
                        tp = pps.tile([128, 128], F32, tag="ptp", space="PSUM")
                        nc.tensor.transpose(
                            out=tp[:], in_=nt_in[:, k * IF:(k + 1) * IF],
                            identity=ident[:])
                        tps = pp.tile([128, 128], F32, tag="ptps")
                        nc.vector.tensor_copy(out=tps[:], in_=tp[:])
                        nc.tensor.matmul(
                            out=pch[:, k * OF:(k + 1) * OF],
                            lhsT=tps[:], rhs=wn[:], start=True, stop=True)
                    pout = pp.tile([128, 4 * OF], F32, tag="pout")
                    nc.vector.tensor_tensor(
                        out=pout[:], in0=pch[:], in1=bb[:], op=mybir.AluOpType.add)
                    if r0 < HS:
                        gt, gr = GLO, 1 + r0
                    else:
                        gt, gr = GHI, 1 + (r0 - HS)
                    wrows = min(512, gt.shape[0] - gr)
                    wrows -= wrows % 4
                    wp = wrows // 4
                    nc.sync.dma_start(
                        out=gt[gr:gr + wrows, 0:OF].rearrange(
                            "(p k) f -> p k f", p=wp),
                        in_=pout[:wp, :])

            # ---- phase B: direct = C + P[src]; scatter-add -------------
            with (
                tc.tile_pool(name="bpool", bufs=2) as bp,
                tc.tile_pool(name="bpsum", bufs=2, space="PSUM") as bps,
                tc.tile_pool(name="bqpsum", bufs=4, space="PSUM") as bqs,
            ):
                nq = (KJB * OF) // 512
                ncol = SUPB // 16
                for s in range(NSUPB):
                    e0 = s * SUPB
                    eft = bp.tile([128, KJB * EFD], F32, tag="eft")
                    nc.sync.dma_start(
                        out=eft[:],
                        in_=ef[e0:e0 + SUPB, :].rearrange("(p j) f -> p j f", p=128))
                    efT = bp.tile([128, KJB * EFD], F32, tag="efT")
                    for m in range(KJB // 2):
                        tp = bps.tile([128, 128], F32, tag="btp", space="PSUM")
                        nc.tensor.transpose(
                            out=tp[:], in_=eft[:, m * 128:(m + 1) * 128],
                            identity=ident[:])
                        nc.vector.tensor_copy(
                            out=efT[:, m * 128:(m + 1) * 128], in_=tp[:])
                    pslo = bp.tile([128, KJB * OF], F32, tag="pslo")
                    pshi = bp.tile([128, KJB * OF], F32, tag="pshi")
                    for t, src_idx, gt, tg in (
                        (pslo, sBlo, GLO, "iblo"), (pshi, sBhi, GHI, "ibhi")):
                        it = bp.tile([128, ncol], I16, tag=tg)
                        nc.sync.dma_start(
                            out=it[:], in_=src_idx[:, s * ncol:(s + 1) * ncol])
                        t3 = t[:].rearrange("p (t f) -> p t f", f=OF)
                        for c in range(SUPB // 1024):
                            nc.gpsimd.dma_gather(
                                t3[:, c * 8:(c + 1) * 8, :],
                                gt[:, 0:OF], it[:, c * 64:(c + 1) * 64],
                                1024, 1024, OF, elem_step=2 * OF)
                    ps = pslo
                    nc.vector.tensor_tensor(
                        out=ps[:], in0=pslo[:], in1=pshi[:], op=mybir.AluOpType.add)
                    direct = bp.tile([128, KJB * OF], F32, tag="direct")
                    d3 = direct[:].rearrange("p (t f) -> p t f", f=OF)
                    ps3 = ps[:].rearrange("p (t f) -> p t f", f=OF)
                    for q in range(nq):
                        cq = bqs.tile([128, 512], F32, tag="cq", space="PSUM")
                        for m in range(4):
                            jp = q * 4 + m
                            nc.tensor.matmul(
                                out=cq[:, m * 128:(m + 1) * 128],
                                lhsT=efT[:, jp * 128:(jp + 1) * 128],
                                rhs=wbd[:], start=True, stop=True)
                        cq3 = cq[:].rearrange("p (t f) -> p t f", f=OF)
                        nc.vector.tensor_tensor(
                            out=d3[:, q * 8:(q + 1) * 8, :], in0=cq3[:],
                            in1=ps3[:, q * 8:(q + 1) * 8, :], op=mybir.AluOpType.add)
                    for tab, idx_src, tag in ((TLO, mMlo, "imlo"), (THI, mMhi, "imhi")):
                        it = bp.tile([128, ncol], I16, tag=tag)
                        nc.sync.dma_start(
                            out=it[:], in_=idx_src[:, s * ncol:(s + 1) * ncol])
                        for c in range(SUPB // 1024):
                            nc.gpsimd.dma_scatter_add(
                                tab[:], d3[:, c * 8:(c + 1) * 8, :],
                                it[:, c * 64:(c + 1) * 64], 1024, 1024, OF)

                # spill rounds: fold aux rows back into their true rows
                AUX = cfg.aux
                for tab, h in ((TLO, "lo"), (THI, "hi")):
                    aux_base = tab.shape[0] - AUX
                    at = bp.tile([128, (AUX // 128) * OF], F32, tag="aux" + h)
                    nc.sync.dma_start(
                        out=at[:].rearrange("p (k f) -> p k f", f=OF),
                        in_=tab[aux_base:, :].rearrange("(k p) f -> p k f", p=128))
                    a3 = at[:].rearrange("p (t f) -> p t f", f=OF)
                    off = 0
                    for r, cap in enumerate(cfg.spill_caps):
                        it = bp.tile([128, cap // 16], I16, tag=f"isp{h}{r}")
                        nc.sync.dma_start(out=it[:], in_=spills[(h, r)][:])
                        co = 0
                        while co < cap:
                            step = min(1024, cap - co)
                            nc.gpsimd.dma_scatter_add(
                                tab[:],
                                a3[:, (off + co) // 128:(off + co + step) // 128, :],
                                it[:, co // 16:(co + step) // 16],
                                step, step, OF)
                            co += step
                        off += cap

            # ---- AllReduce the two halves (real node rows only) --------
            if cfg.collective:
                nc.gpsimd.collective_compute(
                    "AllReduce", mybir.AluOpType.add,
                    ins=[TLO[0:HS, :]], outs=[SLO[:]], replica_groups=rgroups)
                nc.gpsimd.collective_compute(
                    "AllReduce", mybir.AluOpType.add,
                    ins=[THI[0:NT - HS, :]], outs=[SHI[:]], replica_groups=rgroups)
            else:
                for stab, tab, rows in ((SLO, TLO, HS), (SHI, THI, NT - HS)):
                    zc = 0
                    while zc < rows:
                        st = min(2048, rows - zc)
                        p = st // 16
                        nc.sync.dma_start(
                            out=stab[zc:zc + st, :].rearrange(
                                "(p k) f -> p k f", p=p),
                            in_=tab[zc:zc + st, :].rearrange(
                                "(p k) f -> p k f", p=p))
                        zc += st

            # ---- fold full into G + new_node = relu(full) --------------
            with tc.tile_pool(name="rpool", bufs=3) as rp:
                for half, (stab, gtab, base) in enumerate(
                        ((SLO, GLO, 0), (SHI, GHI, HS))):
                    real = HS if half == 0 else (N - HS)
                    r = 0
                    while r < real:
                        left = real - r
                        if left >= 16:
                            step = min(2048, left - left % 16)
                            p = step // 16
                        else:
                            step, p = left, left
                        w = (step // p) * OF
                        ft = rp.tile([128, 1024], F32, tag="fchunk")
                        nc.sync.dma_start(
                            out=ft[:p, :w],
                            in_=stab[r:r + step, :].rearrange(
                                "(p k) f -> p k f", p=p))
                        nc.sync.dma_start(
                            out=gtab[1 + r:1 + r + step, OF:].rearrange(
                                "(p k) f -> p k f", p=p),
                            in_=ft[:p, :w])
                        rl = rp.tile([128, 1024], F32, tag="rchunk")
                        nc.scalar.activation(
                            out=rl[:p, :w], in_=ft[:p, :w],
                            func=mybir.ActivationFunctionType.Relu)
                        nc.sync.dma_start(
                            out=new_node[base + r:base + r + step, :].rearrange(
                                "(p k) f -> p k f", p=p),
                            in_=rl[:p, :w])
                        r += step

            # ---- phase C: outputs --------------------------------------
            with (
                tc.tile_pool(name="cpool", bufs=2) as cp,
                tc.tile_pool(name="cpsum", bufs=2, space="PSUM") as cps,
                tc.tile_pool(name="cqpsum", bufs=4, space="PSUM") as cqs,
            ):
                nq = (KJC * OF) // 512
                ncol = SUPC // 16
                for s in range(NSUPC):
                    e0 = s * SUPC
                    eft = cp.tile([128, KJC * EFD], F32, tag="ceft")
                    nc.sync.dma_start(
                        out=eft[:],
                        in_=ef[e0:e0 + SUPC, :].rearrange("(p j) f -> p j f", p=128))
                    efT = cp.tile([128, KJC * EFD], F32, tag="cefT")
                    for m in range(KJC // 2):
                        tp = cps.tile([128, 128], F32, tag="ctp", space="PSUM")
                        nc.tensor.transpose(
                            out=tp[:], in_=eft[:, m * 128:(m + 1) * 128],
                            identity=ident[:])
                        nc.vector.tensor_copy(
                            out=efT[:, m * 128:(m + 1) * 128], in_=tp[:])
                    gath = {}
                    for nm, idx_src, gt in (
                        ("slo", sClo, GLO), ("shi", sChi, GHI),
                        ("dlo", dClo, GLO), ("dhi", dChi, GHI),
                    ):
                        t = cp.tile([128, KJC * 2 * OF], F32, tag="g" + nm)
                        it = cp.tile([128, ncol], I16, tag="i" + nm)
                        nc.sync.dma_start(
                            out=it[:], in_=idx_src[:, s * ncol:(s + 1) * ncol])
                        t3 = t[:].rearrange("p (t f) -> p t f", f=2 * OF)
                        for c in range(SUPC // 1024):
                            nc.gpsimd.dma_gather(
                                t3[:, c * 8:(c + 1) * 8, :],
                                gt[:], it[:, c * 64:(c + 1) * 64],
                                1024, 1024, 2 * OF)
                        gath[nm] = t
                    gs, gd = gath["slo"], gath["dlo"]
                    nc.vector.tensor_tensor(
                        out=gs[:], in0=gath["slo"][:], in1=gath["shi"][:],
                        op=mybir.AluOpType.add)
                    nc.vector.tensor_tensor(
                        out=gd[:], in0=gath["dlo"][:], in1=gath["dhi"][:],
                        op=mybir.AluOpType.add)
                    gs3 = gs[:].rearrange("p (t f) -> p t f", f=2 * OF)
                    gd3 = gd[:].rearrange("p (t f) -> p t f", f=2 * OF)
                    nd = cp.tile([128, KJC * OF], F32, tag="nd")
                    nb = cp.tile([128, KJC * OF], F32, tag="nb")
                    nd3 = nd[:].rearrange("p (t f) -> p t f", f=OF)
                    nb3 = nb[:].rearrange("p (t f) -> p t f", f=OF)
                    for q in range(nq):
                        cq = cqs.tile([128, 512], F32, tag="ccq", space="PSUM")
                        for m in range(4):
                            jp = q * 4 + m
                            nc.tensor.matmul(
                                out=cq[:, m * 128:(m + 1) * 128],
                                lhsT=efT[:, jp * 128:(jp + 1) * 128],
                                rhs=wbd[:], start=True, stop=True)
                        cq3 = cq[:].rearrange("p (t f) -> p t f", f=OF)
                        sl = slice(q * 8, (q + 1) * 8)
                        nc.vector.tensor_tensor(
                            out=nd3[:, sl, :], in0=gs3[:, sl, OF:],
                            in1=cq3[:], op=mybir.AluOpType.subtract)
                        nc.vector.tensor_tensor(
                            out=nd3[:, sl, :], in0=nd3[:, sl, :],
                            in1=gd3[:, sl, 0:OF], op=mybir.AluOpType.subtract)
                        nc.vector.tensor_tensor(
                            out=nb3[:, sl, :], in0=gd3[:, sl, OF:],
                            in1=cq3[:], op=mybir.AluOpType.subtract)
                        nc.vector.tensor_tensor(
                            out=nb3[:, sl, :], in0=nb3[:, sl, :],
                            in1=gs3[:, sl, 0:OF], op=mybir.AluOpType.subtract)
                    nc.sync.dma_start(
                        out=new_direct[e0:e0 + SUPC, :].rearrange(
                            "(p j) f -> p j f", p=128),
                        in_=nd[:])
                    nc.sync.dma_start(
                        out=new_backward[e0:e0 + SUPC, :].rearrange(
                            "(p j) f -> p j f", p=128),
                        in_=nb[:])

    nc.compile()
    return nc


# ======================= host-side preparation =========================

CALL = 1024                 # tokens per dma_gather/dma_scatter_add call


def _tok_order(S):
    """token q -> edge offset within a super of size S, 1024-token calls.

    Call chunk c covers j-blocks [8c, 8c+8); within a chunk token
    u = jj*128 + p maps to edge p*(S//128) + 8c + jj."""
    q = np.arange(S)
    c, r = q // CALL, q % CALL
    jj, p = r // 128, r % 128
    return p * (S // 128) + c * 8 + jj


def _wrap16(a):
    """[NSUP, S] token-ordered values -> [128, total//16] int16 wrapped."""
    ns, S = a.shape
    w = a.reshape(ns, S // 16, 16).swapaxes(1, 2).reshape(ns, 16, S // 16)
    w = np.concatenate(list(w), axis=1)
    return np.tile(w.astype(np.int16), (8, 1)).copy()


def prep_core_inputs(cfg: Cfg, node_feats, W, b, ef_shard, src_shard, dst_shard):
    N, HS, NT = cfg.n_nodes, cfg.hsplit, cfg.nt
    SUPB, SUPC, ECP, EC = cfg.supb, cfg.supc, cfg.ecp, len(src_shard)

    efp = np.zeros((ECP, 64), dtype=np.float32)
    efp[:EC] = ef_shard
    srcp = np.full((ECP,), N, dtype=np.int64)
    srcp[:EC] = src_shard
    dstp = np.full((ECP,), N, dtype=np.int64)
    dstp[:EC] = dst_shard

    def gather_idx(vals, S):
        tok = vals.reshape(-1, S)[:, _tok_order(S)]
        lo = np.where(tok < HS, tok + 1, 0).reshape(-1, CALL)
        hi = np.where(tok >= HS, tok - HS + 1, 0).reshape(-1, CALL)
        return _wrap16(lo), _wrap16(hi)

    sBlo, sBhi = gather_idx(srcp, SUPB)
    sClo, sChi = gather_idx(srcp, SUPC)
    dClo, dChi = gather_idx(dstp, SUPC)

    # ---- collision-free scatter schedule (phase-B token order) ---------
    NSUPB = cfg.nsupb
    dst_tok = dstp.reshape(NSUPB, SUPB)[:, _tok_order(SUPB)]
    flat = dst_tok.ravel()
    valid = flat < N                      # pad tokens go straight to trash
    n = flat.size
    sup_id = np.repeat(np.arange(n // CALL), CALL)   # per-call dedup
    key_d = np.where(valid, flat, -1)
    order = np.lexsort((np.arange(n), key_d, sup_id))
    sd, ss = key_d[order], sup_id[order]
    newg = np.r_[True, (ss[1:] != ss[:-1]) | (sd[1:] != sd[:-1])]
    pos = np.arange(n)
    rank_sorted = pos - np.maximum.accumulate(np.where(newg, pos, 0))
    rank = np.empty(n, np.int64)
    rank[order] = rank_sorted
    spill_mask = valid & (rank > 0)

    # global spill round per dst
    sp_pos = np.nonzero(spill_mask)[0]
    sp_dst = flat[sp_pos]
    o2 = np.lexsort((sp_pos, sp_dst))
    ng2 = np.r_[True, sp_dst[o2][1:] != sp_dst[o2][:-1]]
    p2 = np.arange(sp_pos.size)
    r2 = p2 - np.maximum.accumulate(np.where(ng2, p2, 0))
    round_of = np.empty(sp_pos.size, np.int64)
    round_of[o2] = r2

    caps = cfg.spill_caps
    if sp_pos.size and round_of.max() >= len(caps):
        raise RuntimeError(
            f"spill rounds overflow: need {round_of.max() + 1} > {len(caps)}")

    thi_node_rows = NT - HS
    trash_lo = HS + (np.arange(n) % 128)
    trash_hi = thi_node_rows + (np.arange(n) % 128)
    mlo = np.where(valid & (flat < HS) & (rank == 0), flat, trash_lo)
    mhi = np.where(valid & (flat >= HS) & (rank == 0), flat - HS, trash_hi)
    aux_lo_base = HS + 128
    aux_hi_base = thi_node_rows + 128
    cap_off = np.concatenate([[0], np.cumsum(caps)])
    sp_half_hi = sp_dst >= HS
    sp_arrays = {}
    for h, hi_sel, aux_base, node_base in (
        ("lo", False, aux_lo_base, 0), ("hi", True, aux_hi_base, HS)):
        for r, cap in enumerate(caps):
            sel = (sp_half_hi == hi_sel) & (round_of == r)
            cnt = int(sel.sum())
            if cnt > cap:
                raise RuntimeError(
                    f"spill capacity overflow: half {h} round {r}: {cnt} > {cap}")
            slots = np.nonzero(sel)[0]
            aux_rel = cap_off[r] + np.arange(cnt)
            if h == "lo":
                mlo[sp_pos[slots]] = aux_base + aux_rel
            else:
                mhi[sp_pos[slots]] = aux_base + aux_rel
            arr = (aux_base + cap_off[r] + np.arange(cap)).astype(np.int64)
            arr[:cnt] = sp_dst[slots] - node_base
            sp_arrays[(h, r)] = _wrap16(arr.reshape(1, cap))

    assert mlo.max() < cfg.tlo_rows and mhi.max() < cfg.thi_rows

    out = {
        "node_feats": np.ascontiguousarray(node_feats, dtype=np.float32),
        "W": np.ascontiguousarray(W, dtype=np.float32),
        "b4": np.tile(b.astype(np.float32), (128, 4)).copy(),
        "ef": efp,
        "sBlo": sBlo, "sBhi": sBhi,
        "sClo": sClo, "sChi": sChi, "dClo": dClo, "dChi": dChi,
        "mMlo": _wrap16(mlo.reshape(-1, CALL)),
        "mMhi": _wrap16(mhi.reshape(-1, CALL)),
    }
    for (h, r), arr in sp_arrays.items():
        out[f"sp_{h}_{r}"] = arr
    return out


_PROG_CACHE: dict = {}


def _get_program(cfg: Cfg) -> bass.Bass:
    if cfg not in _PROG_CACHE:
        _PROG_CACHE[cfg] = build_program(cfg)
    return _PROG_CACHE[cfg]


def kernel(node_feats, edge_feats, W, b, src, dst, _trace=False):
    from concourse.bass_utils import run_bass_kernel_spmd

    node_feats = np.asarray(node_feats)
    edge_feats = np.asarray(edge_feats)
    W = np.asarray(W)
    b = np.asarray(b)
    src = np.asarray(src)
    dst = np.asarray(dst)

    cfg = Cfg(n_nodes=node_feats.shape[0], n_edges=edge_feats.shape[0])
    nc = _get_program(cfg)
    ec = cfg.ec
    in_maps = [
        prep_core_inputs(
            cfg, node_feats, W, b,
            edge_feats[c * ec:(c + 1) * ec],
            src[c * ec:(c + 1) * ec],
            dst[c * ec:(c + 1) * ec],
        )
        for c in range(cfg.n_cores)
    ]
    res = None
    for attempt in range(3):
        try:
            res = run_bass_kernel_spmd(
                nc, in_maps, core_ids=list(range(cfg.n_cores)), trace=_trace)
            break
        except Exception:
            if attempt == 2:
                raise
            import time as _time
            _time.sleep(5)
    outs = res.results
    new_direct = np.concatenate(
        [outs[c]["new_direct"][:ec] for c in range(cfg.n_cores)])
    new_backward = np.concatenate(
        [outs[c]["new_backward"][:ec] for c in range(cfg.n_cores)])
    new_node = outs[0]["new_node"]
    kernel.last_results = res
    return (new_node, new_direct, new_backward)
